# revision 14
# baseline (speedup 1.0000x reference)
"""Trainium2 Bass kernel for the relational GCN layer (gnn_message_passing).

Math (from the reference):
    out[n, e, i] = sum_k sum_m sum_d adj[n, m, k] * x[m, d, (i-k)%4] * W[d, e, k]

Factored for the PE (contraction dim must sit on SBUF partitions):
    X4[m, f]   = x.reshape(4096, 128)            with f = d*4 + j
    G_k[f, n]  = sum_m X4[m, f] * adj[n, m, k]   (the big 256 MB contraction)
    outT[c, n] = sum_k sum_f Wbig[f, k, c] * G_k[f, n]   with c = e*4 + i
    Wbig[d*4+j, k, e*4+i] = W[d, e, k] if j == (i-k)%4 else 0

Precision: the rel-err budget is 2e-2, so adj is streamed as a SINGLE fp8
pass instead of fp16 hi/lo pairs.  adj ~ U[0,1) is centered and scaled
(16*(adj-0.5)) and quantized to float8_e3m4 — on uniform data e3m4 acts as
a ~6.2-bit uniform quantizer, and centering removes the mean so the exact
rank-1 term 0.5*sum_m x4[m,f] (host-computed in f64) is folded back in as
a per-partition bias on the output.  x rides as fp16 (near-exact), G and
Wbig/16 as bf16.  Measured end-to-end rel err ~7e-3 (threshold 2e-2).

This cuts HBM traffic 3.5x (32 MB -> 9.2 MB per core) and PE columns 3x
(one product per (chunk, k) instead of three) vs the hi/lo fp16 version.

Sharding: 1D over the node (row) dim of adj/out — core c owns rows
[c*512, (c+1)*512).  x, Wbig and the bias are replicated.  adj is packed
on the host into centered e3m4 tiles laid out exactly as the PE streams
them ([m-partition, (k, n)-free], contiguous per partition per DMA).
"""

import numpy as np
import ml_dtypes

N_CORES = 8
NODES = 4096
N_PER_CORE = NODES // N_CORES          # 512
F = 128                                # d*4+j
C = 128                                # e*4+i
MB = 32                                # m-chunks of 128 (4096 / 128)
R = 4
MB_COLS = R * N_PER_CORE               # (k, nn) = 2048 fp8 bytes per m-chunk
# Adj chunk-DMA group sizes: fine-grained while the DMA subsystem ramps up
# (so the first matmuls start ASAP), coarse later (fewer DMA instructions =
# fewer queues/semaphores = shorter NEFF epilogue, which resets each one).
GROUP_SIZES = [1, 1, 1, 1, 2, 2, 4, 4, 4, 4, 4, 4]
GMAX = max(GROUP_SIZES)
assert sum(GROUP_SIZES) == MB
XP0 = 8                                # x chunks 0..7 lead the SP ring
ADJ_SCALE = 16.0                       # fp8 stores 16*(adj-0.5)

_PATCHED = False
_PROG = None


def _patch_tile_drain():
    """This container's walrus build rejects >2 sync waits on one Drain;
    split the Tile end-of-context drain into one single-wait drain per proc
    (semantically identical: the SP engine observes each clock lane in
    sequence before the barrier)."""
    global _PATCHED
    if _PATCHED:
        return
    from concourse.tile import TileContext
    from concourse.vector_clock import ScopedClock, VectorClock
    from concourse.tile_scheduler import N_PROCS

    def _split_drain_and_barrier(self, tick_clock, wait_clock):
        g = tick_clock.global_clock
        for p in range(N_PROCS):
            if g[p] > 0:
                d = self.nc.sync.drain()
                pc = VectorClock([g[q] if q == p else 0 for q in range(N_PROCS)])
                wait_clock.add_sem_waits(d.ins, ScopedClock({None: pc}))
        self.nc.all_engine_barrier()
        assert self.sems is not None
        popped = self.nc._tile_sem_poison_stack.pop()
        assert popped is self._sem_poison
        self.nc.clear_and_free_semaphores(list(self.sems.allocated().values()))
        self.nc.all_engine_barrier()

    TileContext._drain_and_barrier = _split_drain_and_barrier
    _PATCHED = True


def _split_sync_waits(bir_bytes, max_waits=1):
    """This container's walrus build rejects instructions carrying more than
    ~2 sync waits.  Hoist all but one wait of any instruction onto standalone
    EventSemaphore instructions on the same engine immediately before it —
    the engine then observes the semaphores sequentially, which is
    semantically identical."""
    import json
    j = json.loads(bir_bytes)

    # normalize all debug records (top-level debug_table entries and inline
    # ant_debug dicts): their traceback/path strings vary by process context
    # and working directory, which would defeat the content-addressed NEFF
    # cache
    def scrub(o):
        if isinstance(o, dict):
            if "ant_traceback" in o or "filename" in o:
                for key, stub in (("filename", "kernel.py"),
                                  ("kernel_name", "k"), ("ant_traceback", "")):
                    if key in o:
                        o[key] = stub
                if "lineno" in o:
                    o["lineno"] = 0
            for v in o.values():
                scrub(v)
        elif isinstance(o, list):
            for v in o:
                scrub(v)

    scrub(j)
    n_new = 0
    for f in j.get("functions", []):
        for bb in f.get("blocks", []):
            out_insts = []
            for inst in bb.get("instructions", []):
                waits = (inst.get("sync_info") or {}).get("on_wait") or []
                if len(waits) > max_waits:
                    keep = waits[-max_waits:]
                    for w in waits[:-max_waits]:
                        n_new += 1
                        ev = {
                            "engine": inst["engine"],
                            "ins": [],
                            "name": f"{inst['name']}_wsplit{n_new}",
                            "opcode": "EventSemaphore",
                            "outs": [],
                            "sync_info": {"on_update": [], "on_wait": [w]},
                        }
                        if "debug" in inst:
                            ev["debug"] = inst["debug"]
                        out_insts.append(ev)
                    inst["sync_info"]["on_wait"] = keep
                out_insts.append(inst)
            bb["instructions"] = out_insts
    return json.dumps(j).encode()


def _install_neff_cache():
    """The bass_exec compile path bypasses libneuronxla's NEFF cache, so a
    fresh process pays the full ~3 min walrus compile every run.  Add a
    content-addressed cache keyed on the exact BIR bytes."""
    import hashlib, os, shutil
    import concourse.bass_utils as bu
    import concourse.bass2jax as b2j
    if getattr(bu, "_ant_bir_neff_cache", False):
        return
    orig = bu.compile_bir_kernel
    cache_dir = os.path.expanduser("~/.neuron-compile-cache/bass-bir-neff")
    os.makedirs(cache_dir, exist_ok=True)

    def cached(bir_json, tmpdir, neff_name="file.neff"):
        data = bir_json if isinstance(bir_json, bytes) else bir_json.encode()
        key = hashlib.sha256(data).hexdigest()
        cpath = os.path.join(cache_dir, key + ".neff")
        if os.path.exists(cpath):
            dst = os.path.join(tmpdir, neff_name)
            shutil.copy(cpath, dst)
            return dst
        neff = orig(bir_json, tmpdir, neff_name)
        try:
            shutil.copy(neff, cpath)
        except OSError:
            pass
        return neff

    bu.compile_bir_kernel = cached
    b2j.compile_bir_kernel = cached
    bu._ant_bir_neff_cache = True


def _build_program():
    global _PROG
    if _PROG is not None:
        return _PROG
    _patch_tile_drain()
    _install_neff_cache()
    import concourse.bass as bass
    import concourse.mybir as mybir
    from concourse.tile import TileContext

    f32 = mybir.dt.float32
    f16 = mybir.dt.float16
    bf16 = mybir.dt.bfloat16
    f8 = mybir.dt.float8e3
    nc = bass.Bass()
    # adjt[mb, mp, (k, nn)]: e3m4 of 16*(adj[n0+nn, mb*128+mp, k] - 0.5)
    adjt = nc.dram_tensor("adjt", [MB, 128, MB_COLS], f8, kind="ExternalInput")
    # xt[mp, mb, f]: fp16 of x.reshape(4096, 128)[mb*128+mp, f]
    xt = nc.dram_tensor("xt", [128, MB, F], f16, kind="ExternalInput")
    # wt[f, k, c] = Wbig/ADJ_SCALE in bf16
    wt = nc.dram_tensor("wt", [F, R, C], bf16, kind="ExternalInput")
    # bt[c]: exact rank-1 bias sum_k,f (0.5*sum_m x4[m,f]) * Wbig[f,k,c]
    bt = nc.dram_tensor("bt", [C, 1], f32, kind="ExternalInput")
    # outt[h, c, nn]: half-major so each half's store is fully contiguous
    NH = N_PER_CORE // 2
    outt = nc.dram_tensor("outt", [2, C, NH], f32, kind="ExternalOutput")

    with TileContext(nc) as tc:
        with (
            tc.tile_pool(name="const", bufs=1) as cpool,
            tc.tile_pool(name="adj", bufs=16) as apool,
            tc.tile_pool(name="gout", bufs=1) as gpool,
            tc.tile_pool(name="psum", bufs=1, space="PSUM") as ppool,
        ):
            Copy = mybir.ActivationFunctionType.Copy
            # HAM warmup: a single accumulation chain of dummy matmuls,
            # long enough (>3.4us busy) to flip the PE clock-gate to 8/8
            # while the first DMAs land.
            warm = cpool.tile([128, F], f16)
            nc.vector.memset(warm[:, :], 0.0)
            wps = ppool.tile([128, 64], f32, tag="warm")
            NWARM = 70
            for i in range(NWARM):
                nc.tensor.matmul(wps[:, :], lhsT=warm[:, :], rhs=warm[:, :64],
                                 start=(i == 0), stop=(i == NWARM - 1))

            # x chunks 0..7 lead the SP ring (they gate the first matmuls);
            # Wbig + bias ride the gpsimd (SWDGE) ring — tail-only data
            xsb = cpool.tile([128, MB, F], f16)
            nc.sync.dma_start(out=xsb[:, :XP0, :], in_=xt[:, :XP0, :])
            wsb = cpool.tile([F, R, C], bf16)
            nc.gpsimd.dma_start(out=wsb[:, :, :], in_=wt[:, :, :])
            bsb = cpool.tile([C, 1], f32)
            nc.gpsimd.dma_start(out=bsb[:, :], in_=bt[:, :])

            # Stall the ACT ring behind x piece 0 so the SP ring's first
            # chunks own the (slowly ramping) DMA subsystem in consumption
            # order; doubles as the ACT-table pre-load so the tail's
            # activation casts don't pay the ~1.3us table DMA.
            scr = cpool.tile([128, 1], f16)
            nc.scalar.activation(scr[:, :], xsb[:, 0, :1], Copy)

            gps = [ppool.tile([F, N_PER_CORE], f32, tag=f"g{k}", name=f"gps{k}")
                   for k in range(R)]

            def rhs(adjsb, a, k):
                off = k * N_PER_CORE
                return adjsb[:, a, off:off + N_PER_CORE]

            mb0 = 0
            for gi, gsz in enumerate(GROUP_SIZES):
                adjsb = apool.tile([128, GMAX, MB_COLS], f8, tag="adjsb")
                # first 4 groups stream on the SP ring in consumption
                # order; later groups alternate ACT/SP
                if gi < 4:
                    dma_eng = nc.sync
                else:
                    dma_eng = nc.scalar if gi % 2 == 0 else nc.sync
                if gsz == 1:
                    dma_eng.dma_start(out=adjsb[:, 0, :], in_=adjt[mb0])
                else:
                    dma_eng.dma_start(
                        out=adjsb[:, :gsz, :],
                        in_=adjt[mb0:mb0 + gsz].rearrange("a p c -> p a c"))
                if gi == 4:
                    # rest of x behind the ACT ring's first adj group
                    nc.scalar.dma_start(out=xsb[:, XP0:, :],
                                        in_=xt[:, XP0:, :])
                for a in range(gsz):
                    mb = mb0 + a
                    lhsT = xsb[:, mb, :]
                    # serpentine k avoids a psum bank jump at boundaries
                    ks = range(R) if mb % 2 == 0 else range(R - 1, -1, -1)
                    for k in ks:
                        nc.tensor.matmul(gps[k][:, :], lhsT=lhsT,
                                         rhs=rhs(adjsb, a, k),
                                         start=(mb == 0), stop=(mb == MB - 1))
                mb0 += gsz

            # Tail: PSUM G -> SBUF bf16 in 8 (k, half) pieces split across
            # DVE and ACT so the casts run in parallel, each its own tile so
            # the stage-2 matmuls chase individual casts (not the full set);
            # k ordered as the last chunk's matmuls complete.
            kcopy = list(range(R - 1, -1, -1)) if (MB - 1) % 2 else list(range(R))
            gkh = {}
            for k in kcopy:
                for h in range(2):
                    gkh[(k, h)] = gpool.tile([F, NH], bf16, tag=f"g{k}{h}",
                                             name=f"gkh{k}{h}")
                nc.vector.tensor_copy(gkh[(k, 0)][:, :], gps[k][:, :NH])
                nc.scalar.activation(gkh[(k, 1)][:, :], gps[k][:, NH:], Copy)

            # finals: both halves' matmuls chase the casts k-by-k; bias is
            # folded into the PSUM->SBUF copy; the two halves ship on
            # different HWDGE rings
            osb = [gpool.tile([C, NH], f32, tag=f"osb{h}", name=f"osb{h}")
                   for h in range(2)]
            ops = [ppool.tile([C, NH], f32, tag=f"out{h}", name=f"ops{h}")
                   for h in range(2)]
            for ki, k in enumerate(kcopy):
                for h in range(2):
                    nc.tensor.matmul(ops[h][:, :], lhsT=wsb[:, k, :],
                                     rhs=gkh[(k, h)][:, :],
                                     start=(ki == 0), stop=(ki == R - 1))
            for h, eng in ((0, nc.sync), (1, nc.scalar)):
                nc.vector.tensor_scalar_add(osb[h][:, :], ops[h][:, :],
                                            bsb[:, :])
                eng.dma_start(out=outt[h], in_=osb[h][:, :])

    _orig_to_json = nc.to_json_bytes
    nc.to_json_bytes = lambda: _split_sync_waits(_orig_to_json())

    _PROG = nc
    return nc


def _pack_adj(adj):
    """adj [4096, 4096, 4] f32 -> per-core [MB, 128, MB_COLS] e3m4 with
    adjt[c][mb, mp, (k, nn)] = e3m4(16*(adj[c*512+nn, mb*128+mp, k] - 0.5))."""
    A = adj.reshape(N_CORES, N_PER_CORE, MB, 128, R)
    At = np.ascontiguousarray(A.transpose(0, 2, 3, 4, 1))  # [c,mb,mp,k,nn]
    q = ((At - np.float32(0.5)) * np.float32(ADJ_SCALE)).astype(
        ml_dtypes.float8_e3m4)
    return q.reshape(N_CORES, MB, 128, MB_COLS)


def _prepare_in_maps(x, adj, weight):
    x = np.ascontiguousarray(np.asarray(x), dtype=np.float32)
    adj = np.ascontiguousarray(np.asarray(adj), dtype=np.float32)
    weight = np.asarray(weight).astype(np.float64)

    x4 = x.reshape(NODES, F)                               # [m, f], f = d*4+j
    xt = np.ascontiguousarray(
        x4.reshape(MB, 128, F).transpose(1, 0, 2)).astype(np.float16)

    wbig = np.zeros((F, R, C), np.float64)                 # [f, k, c]
    for k in range(R):
        for i in range(R):
            j = (i - k) % R
            wbig[j::R, k, i::R] = weight[:, :, k]
    wt = (wbig / ADJ_SCALE).astype(ml_dtypes.bfloat16)

    bias_f = 0.5 * x4.astype(np.float64).sum(axis=0)       # [f]
    b_out = np.einsum('f,fkc->c', bias_f, wbig)            # [c]
    bt = np.ascontiguousarray(b_out.astype(np.float32).reshape(C, 1))

    adjq = _pack_adj(adj)
    return [{"adjt": adjq[c], "xt": xt, "wt": wt, "bt": bt}
            for c in range(N_CORES)]


def _assemble_out(results):
    outt = np.stack([r["outt"] for r in results])          # [8, 2, 128, 256]
    out = outt.reshape(N_CORES, 2, 32, R, N_PER_CORE // 2) # [c, h, e, i, nn]
    out = out.transpose(0, 1, 4, 2, 3).reshape(NODES, 32, R)
    return np.ascontiguousarray(out)


def kernel(x, adj, weight):
    import os
    # the bass runner reaches the NeuronCores through the axon PJRT proxy;
    # make sure jax can initialize that platform (harmless if already set)
    plats = os.environ.get("JAX_PLATFORMS", "")
    if "axon" not in plats:
        os.environ["JAX_PLATFORMS"] = "axon,cpu" if not plats else f"axon,{plats}"
    nc = _build_program()
    in_maps = _prepare_in_maps(x, adj, weight)
    from concourse.bass_utils import run_bass_kernel_spmd
    res = run_bass_kernel_spmd(nc, in_maps, core_ids=list(range(N_CORES)))
    return _assemble_out(res.results)
